# revision 30
# baseline (speedup 1.0000x reference)
"""BasesDecomposition (R-GCN style) message passing kernel for Trainium2.

Strategy (8 NeuronCores, SPMD — one program, per-core data):
  - Nodes sharded by row: core c owns targets [c*NL, (c+1)*NL).
  - Edges symmetrized on host, partitioned by target-owner core, then by
    target BLOCK-GROUP (G groups of ~25 blocks), then by relation (each
    (group, rel) run padded to a cross-core-uniform multiple of 128 so
    the chunk -> W_r schedule is program-static); within a (group, rel)
    run, edges are ordered by target block.
  - Per-relation weights W_r = sum_b rbw[r, b] * bases[b] (host, bf16).
  - Host builds, per core, a pre-transposed pre-scaled source table
    xsT[d, pos] = x[src_e, d] * ew_e (bf16, zeros at padding).
  - Messages round-trip through per-GROUP DRAM tensors md[g] so that
    aggregation for group g's blocks can start as soon as group g's
    messages are written — overlapping with message computation for
    later groups (keeps gpsimd/vector/tensor busy under the DMA wall
    and the PE warm).
  - Phase 1: per 128-edge chunk, one bf16 matmul msg = xsT_chunk^T @ W_r
    (PSUM f32), cast-copied (vector/scalar alternate) into a staging
    tile of MDG chunks, then one DMA writes the message rows to md[g].
    A host-side permutation maps xsT columns to md rows so each SBUF
    partition supplies 16 consecutive md rows -> 4KB write descriptors.
    xsT loads go on the scalar HWDGE ring, md writes on the sync ring,
    so loads and stores overlap.
  - Phase 2 (one 128-target block per iteration): one indirect gather
    pulls SL consecutive md rows for each of IV interval starts covering
    the block's per-relation runs; one tensor_tensor(is_equal) against
    an iota constant builds the 0/1 scatter matrix T (layout p (e s) so
    all operands have packed last dims -> DVE 2x mode eligible); the
    self-loop matmul plus SL accumulating matmuls out^T += mg_j^T @ T_j
    run in PSUM; the block is cast to bf16 and written out.
  - Host reassembles out (f32) from the per-core outT (bf16) blocks.
"""

import numpy as np
import ml_dtypes

import concourse.bass as bass
import concourse.bacc as bacc
import concourse.tile as tile
import concourse.mybir as mybir
from concourse.bass_utils import run_bass_kernel_spmd

F32 = mybir.dt.float32
BF16 = mybir.dt.bfloat16
I32 = mybir.dt.int32

NCORE = 8
R = 32           # num_relations (relation id R is the self-loop row of rbw)
G = 3            # target-block groups (md pipelining granularity)
GFRAC = (0.0, 0.41, 0.82, 1.0)   # group boundaries as fractions of nblk
LDTOK = 8192     # tokens per xsT load slice
MDG = 32         # chunks per md write group
PS4 = 4          # matmul outputs packed per PSUM bank (one wide copy each)
WPG = 2048       # md rows per write-slice granule
SL_CANDIDATES = (8, 10, 12, 16)
QUOTA = 7        # aggregation blocks interleaved per md write slot
LOOKAHEAD = 2    # gathers kept in flight ahead of their matmuls
BF = ml_dtypes.bfloat16


def _splits(total, step):
    out = []
    off = 0
    while off < total:
        sz = min(step, total - off)
        out.append((off, sz))
        off += sz
    return out


def host_prep(x, node_keep_mask, source, target, edge_type, edge_weights,
              bases, relation_base_weights):
    n, d = x.shape
    assert n % NCORE == 0
    nl = n // NCORE
    nblk = (nl + 127) // 128
    nlp = nblk * 128

    f32 = np.float32
    W = np.einsum("rb,bdo->rdo", relation_base_weights.astype(f32),
                  bases.astype(f32)).astype(f32)  # (R+1, 128, 128)
    wsb_h = np.ascontiguousarray(
        W.transpose(1, 0, 2).reshape(d, (R + 1) * d)).astype(BF)

    src2 = np.concatenate([source, target]).astype(np.int64)
    tgt2 = np.concatenate([target, source]).astype(np.int64)
    et2 = np.concatenate([edge_type, edge_type]).astype(np.int64)
    ew2 = np.concatenate([edge_weights, edge_weights]).astype(f32)

    owner = tgt2 // nl
    tloc = tgt2 - owner * nl
    blk = tloc // 128
    tin = (tloc - blk * 128).astype(f32)

    # block-group boundaries (G groups of consecutive blocks)
    gb = np.round(np.array(GFRAC) * nblk).astype(np.int64)
    grp = np.searchsorted(gb, blk, side="right") - 1

    # per-(core, group, rel) counts; per-(group, rel) run size uniform
    # across cores, rounded to 256 (chunk purity + the 2-row write perm
    # requires 256-row granules to stay within one relation region).
    cgr = (owner * G + grp) * R + et2
    cnt = np.bincount(cgr, minlength=NCORE * G * R).reshape(NCORE, G, R)
    Gr = ((cnt.max(axis=0) + 255) // 256) * 256          # (G, R)
    glen = ((Gr.sum(axis=1) + WPG - 1) // WPG) * WPG      # (G,) md rows/group
    gxoff = np.concatenate([[0], np.cumsum(glen)[:-1]])   # xsT col base/group
    ep1 = int(glen.sum())

    # group-local relation region starts
    starts_gr = np.cumsum(Gr, axis=1) - Gr                # (G, R) local rows

    # per-group chunk -> relation schedule (tail chunks: rel 0, zero data)
    rel_of_chunk = []
    for g in range(G):
        rc = np.repeat(np.arange(R), Gr[g] // 128)
        rc = np.concatenate([rc, np.zeros(glen[g] // 128 - len(rc),
                                          np.int64)])
        rel_of_chunk.append(tuple(int(r) for r in rc))

    # rank within (core, group, rel), ordered by target block
    starts_cgr = np.concatenate([[0], np.cumsum(cnt.reshape(-1))[:-1]])
    order = np.lexsort((blk, cgr))
    rank = np.empty(len(cgr), np.int64)
    rank[order] = np.arange(len(cgr)) - starts_cgr[cgr[order]]
    # group-local md row of each edge
    lpos = starts_gr[grp, et2] + rank

    # xsT column of md row: within each 256-row granule, partition p
    # holds rows 2p, 2p+1; row 2p+j comes from chunk j of the pair ->
    # 512B per-partition write descriptors.
    def lpos_to_xst(g, m):
        q = m // 256
        w = m % 256
        return gxoff[g] + q * 256 + (w % 2) * 128 + w // 2

    # per-(core, rel, blk) run lengths and starts within (group, rel)
    crb = (owner * R + et2) * nblk + blk
    cnt_crb = np.bincount(crb, minlength=NCORE * R * nblk).reshape(
        NCORE, R, nblk)
    # run start of block b within its (group, rel) region: cumsum over
    # blocks inside the group
    run_start = np.zeros_like(cnt_crb)
    cs = np.cumsum(cnt_crb, axis=2)
    for g in range(G):
        b0, b1 = gb[g], gb[g + 1]
        base = cs[:, :, b0 - 1][:, :, None] if b0 > 0 else 0
        run_start[:, :, b0:b1] = cs[:, :, b0:b1] - cnt_crb[:, :, b0:b1] - base

    # smallest interval stride whose per-(core, block) interval count
    # fits in <= 128 indirect-gather indices
    for SL in SL_CANDIDATES:
        n_iv = int(np.ceil(cnt_crb / SL).sum(axis=1).max())
        if n_iv <= 128:
            break
    else:
        raise AssertionError(f"no SL fits: {n_iv} intervals")
    IV = n_iv
    # per-block interval budget (max over cores) — the program gathers
    # only ivb[b] intervals for block b
    ivb = tuple(int(v) for v in
                np.ceil(cnt_crb / SL).astype(np.int64).sum(axis=1).max(axis=0))

    xf = x.astype(f32)
    keep = node_keep_mask.astype(f32)

    per_core = []
    for c in range(NCORE):
        m = owner == c
        vals = xf[src2[m]] * ew2[m, None]
        xsT_h = np.zeros((128, ep1), BF)
        xsT_h[:, lpos_to_xst(grp[m], lpos[m])] = vals.T.astype(BF)

        # group-local md row -> tin of the edge it holds (-1 if padding)
        row_tin = [np.full(int(glen[g]), -1.0, f32) for g in range(G)]
        for g in range(G):
            mg_ = m & (grp == g)
            row_tin[g][lpos[mg_]] = tin[mg_]

        # phase-2 cover: per block, interval starts covering the runs
        cidx_h = np.zeros((128, nblk), np.int32)
        tcol_h = np.full((128, nblk, SL), -1.0, f32)
        for g in range(G):
            for b in range(int(gb[g]), int(gb[g + 1])):
                p = 0
                for r in range(R):
                    s = int(starts_gr[g, r] + run_start[c, r, b])
                    ln = int(cnt_crb[c, r, b])
                    for off in range(0, ln, SL):
                        st = min(s + off, int(glen[g]) - SL)
                        lo = s + off
                        hi = min(s + off + SL, s + ln)
                        assert p < IV, "cover overflow"
                        cidx_h[p, b] = st
                        sl_rows = np.arange(st, st + SL)
                        use = (sl_rows >= lo) & (sl_rows < hi)
                        tcol_h[p, b, use] = row_tin[g][sl_rows[use]]
                        p += 1
        tcol_h = np.ascontiguousarray(
            tcol_h.reshape(128, nblk * SL)).astype(BF)

        xm = xf[c * nl:(c + 1) * nl] * keep[c * nl:(c + 1) * nl, None]
        xmt_h = np.zeros((128, nlp), BF)
        xmt_h[:, :nl] = xm.T.astype(BF)

        per_core.append({
            "xsT": xsT_h,
            "wsb": wsb_h,
            "xmt": xmt_h,
            "cidx": np.ascontiguousarray(cidx_h),
            "tcol": tcol_h,
        })

    cfg = dict(n=n, nl=nl, nblk=nblk, nlp=nlp, ep1=ep1, SL=SL, IV=IV,
               ivb=ivb,
               gb=tuple(int(v) for v in gb),
               glen=tuple(int(v) for v in glen),
               rel_of_chunk=tuple(rel_of_chunk))
    return per_core, cfg


def build_program(cfg):
    nblk = cfg["nblk"]
    nlp = cfg["nlp"]
    ep1 = cfg["ep1"]
    SL = cfg["SL"]
    ivb = cfg["ivb"]
    gb = cfg["gb"]
    glen = cfg["glen"]
    rel_of_chunk = cfg["rel_of_chunk"]

    nc = bacc.Bacc(None, target_bir_lowering=False, debug=False)

    xsT = nc.declare_dram_parameter("xsT", [128, ep1], BF16, isOutput=False)
    wsb = nc.declare_dram_parameter("wsb", [128, (R + 1) * 128], BF16,
                                    isOutput=False)
    xmt = nc.declare_dram_parameter("xmt", [128, nlp], BF16, isOutput=False)
    cidx = nc.declare_dram_parameter("cidx", [128, nblk], I32, isOutput=False)
    tcol = nc.declare_dram_parameter("tcol", [128, nblk * SL], BF16,
                                     isOutput=False)
    outT = nc.declare_dram_parameter("outT", [128, nlp], BF16, isOutput=True)

    md = [nc.dram_tensor(f"md{g}", [glen[g], 128], BF16) for g in range(G)]

    # colw[p, s*128 + e] = e  (iota over e, tiled SL times)
    colw_d = nc.inline_tensor(
        np.tile(np.arange(128, dtype=np.float32), (128, SL)),
        name="colw_c")

    gxoff = np.concatenate([[0], np.cumsum(glen)[:-1]]).astype(np.int64)

    with tile.TileContext(nc) as tc:
        with tc.tile_pool(name="const", bufs=1) as constp:
            wsb_t = constp.tile([128, (R + 1) * 128], BF16)
            nc.sync.dma_start(out=wsb_t[:], in_=wsb[:])
            xmt_t = constp.tile([128, nlp], BF16)
            nc.sync.dma_start(out=xmt_t[:], in_=xmt[:])
            cidx_t = constp.tile([128, nblk], I32)
            nc.sync.dma_start(out=cidx_t[:], in_=cidx[:])
            tcol_t = constp.tile([128, nblk * SL], BF16)
            nc.sync.dma_start(out=tcol_t[:], in_=tcol[:])
            colw_f = constp.tile([128, SL * 128], F32)
            nc.sync.dma_start(out=colw_f[:], in_=colw_d[:])
            colw = constp.tile([128, SL * 128], BF16)
            nc.vector.tensor_copy(out=colw[:], in_=colw_f[:])

            with (
                tc.tile_pool(name="xt", bufs=3) as xp,
                tc.tile_pool(name="mds", bufs=3) as mdp,
                tc.tile_pool(name="p1ps", bufs=5, space="PSUM") as p1ps,
                tc.tile_pool(name="p2", bufs=10) as p2,
                tc.tile_pool(name="p2t", bufs=10) as p2t,
                tc.tile_pool(name="io", bufs=2) as iop,
                tc.tile_pool(name="p2ps", bufs=3, space="PSUM") as p2ps,
            ):
                state = {"ncopy": 0, "obt": None, "obn": 0, "obb": -1}
                ready = []    # blocks whose md group is complete
                gathered = []  # blocks with gather+T emitted, mms pending

                def flush_ob():
                    # one DMA for the accumulated run of consecutive blocks
                    if state["obn"]:
                        b0 = state["obb"] - state["obn"] + 1
                        nc.scalar.dma_start(
                            out=outT[:, 128 * b0:128 * (b0 + state["obn"])],
                            in_=state["obt"][:, :128 * state["obn"]])
                    state["obt"] = None
                    state["obn"] = 0

                def emit_gather(b, g, iv):
                    mg = p2.tile([128, SL * 128], BF16, tag="mg")
                    nc.gpsimd.indirect_dma_start(
                        out=mg[0:iv, :], out_offset=None, in_=md[g][:, :],
                        in_offset=bass.IndirectOffsetOnAxis(
                            ap=cidx_t[0:iv, b:b + 1], axis=0))
                    tt = p2t.tile([128, SL * 128], BF16, tag="T")
                    tsl = tcol_t[0:iv, b * SL:(b + 1) * SL]
                    tb = tsl.unsqueeze(2).broadcast_to([iv, SL, 128])
                    nc.vector.tensor_tensor(
                        out=tt[0:iv].rearrange("p (s e) -> p s e", e=128),
                        in0=colw[0:iv].rearrange("p (s e) -> p s e", e=128),
                        in1=tb, op=mybir.AluOpType.is_equal)
                    gathered.append((b, iv, mg, tt))

                def emit_mms(b, iv, mg, tt):
                    ps = p2ps.tile([128, 128], F32, tag="acc")
                    nc.tensor.matmul(
                        out=ps[:],
                        lhsT=wsb_t[:, R * 128:(R + 1) * 128],
                        rhs=xmt_t[:, 128 * b:128 * (b + 1)],
                        start=True, stop=False)
                    for j in range(SL):
                        nc.tensor.matmul(
                            out=ps[:],
                            lhsT=mg[0:iv, 128 * j:128 * (j + 1)],
                            rhs=tt[0:iv, 128 * j:128 * (j + 1)],
                            start=False, stop=(j == SL - 1))
                    if state["obn"] and b != state["obb"] + 1:
                        flush_ob()
                    if state["obt"] is None:
                        obt = iop.tile([128, 8 * 128], BF16, tag="ob")
                        state["obt"] = obt
                    k = state["obn"]
                    nc.scalar.copy(out=state["obt"][:, 128 * k:128 * (k + 1)],
                                   in_=ps[:])
                    state["obb"] = b
                    state["obn"] = k + 1
                    if state["obn"] == 8:
                        flush_ob()

                def drain(k):
                    # keep ~LOOKAHEAD gathers in flight ahead of the mms
                    while ready and len(gathered) < LOOKAHEAD:
                        emit_gather(*ready.pop(0))
                    nn = 0
                    while gathered and nn < k:
                        emit_mms(*gathered.pop(0))
                        nn += 1

                # ---------------- Phase 1: messages ----------------
                for g in range(G):
                    roc = rel_of_chunk[g]
                    for loff, lsz in _splits(glen[g], LDTOK):
                        xt = xp.tile([128, LDTOK], BF16, tag="xt")
                        nc.sync.dma_start(
                            out=xt[:, :lsz],
                            in_=xsT[:, gxoff[g] + loff:gxoff[g] + loff + lsz])
                        for goff, gsz in _splits(lsz, MDG * 128):
                            ms = mdp.tile([128, MDG * 128], BF16, tag="ms")
                            for poff, psz in _splits(gsz, PS4 * 128):
                                ps = p1ps.tile([128, PS4 * 128], F32,
                                               tag="ps")
                                for j in range(psz // 128):
                                    ch = (loff + goff + poff) // 128 + j
                                    r = roc[ch]
                                    co = goff + poff + 128 * j
                                    nc.tensor.matmul(
                                        out=ps[:, 128 * j:128 * (j + 1)],
                                        lhsT=xt[:, co:co + 128],
                                        rhs=wsb_t[:, 128 * r:128 * (r + 1)],
                                        start=True, stop=True)
                                state["ncopy"] += 1
                                if state["ncopy"] % 4 == 0:
                                    nc.vector.tensor_copy(
                                        out=ms[:, poff:poff + psz],
                                        in_=ps[:, :psz])
                                else:
                                    nc.scalar.copy(
                                        out=ms[:, poff:poff + psz],
                                        in_=ps[:, :psz])
                            r0 = loff + goff
                            mdv = md[g][r0:r0 + gsz, :]
                            nc.scalar.dma_start(
                                out=mdv.rearrange("(w p j) c -> p w (j c)",
                                                  p=128, j=2),
                                in_=ms[:, :gsz].rearrange("p (w k) -> p w k",
                                                          k=256))
                            drain(QUOTA)
                    # group g's messages fully emitted -> its blocks ready
                    ready.extend((b, g, ivb[b])
                                 for b in range(gb[g], gb[g + 1]))

                # ---------------- Phase 2 tail ----------------
                while ready or gathered:
                    drain(len(ready) + len(gathered))
                flush_ob()

    nc.finalize()
    return nc


_PROGRAM_CACHE = {}


def _get_program(cfg):
    key = tuple(sorted((k, str(v)) for k, v in cfg.items()))
    if key not in _PROGRAM_CACHE:
        _PROGRAM_CACHE[key] = build_program(cfg)
    return _PROGRAM_CACHE[key]


def kernel(x, node_keep_mask, source, target, edge_type, edge_weights,
           bases, relation_base_weights):
    per_core, cfg = host_prep(x, node_keep_mask, source, target, edge_type,
                              edge_weights, bases, relation_base_weights)
    nc = _get_program(cfg)
    res = run_bass_kernel_spmd(nc, per_core, list(range(NCORE)))
    nl = cfg["nl"]
    out = np.empty((cfg["n"], 128), np.float32)
    for c in range(NCORE):
        out[c * nl:(c + 1) * nl] = \
            res.results[c]["outT"][:, :nl].T.astype(np.float32)
    return out


# revision 35
# speedup vs baseline: 1.0033x; 1.0033x over previous
"""BasesDecomposition (R-GCN style) message passing kernel for Trainium2.

Strategy (8 NeuronCores, SPMD — one program, per-core data):
  - Nodes sharded by row: core c owns targets [c*NL, (c+1)*NL).
  - Edges symmetrized on host, partitioned by target-owner core, then by
    target BLOCK-GROUP (G groups of ~25 blocks), then by relation (each
    (group, rel) run padded to a cross-core-uniform multiple of 128 so
    the chunk -> W_r schedule is program-static); within a (group, rel)
    run, edges are ordered by target block.
  - Per-relation weights W_r = sum_b rbw[r, b] * bases[b] (host, bf16).
  - Host builds, per core, a pre-transposed pre-scaled source table
    xsT[d, pos] = x[src_e, d] * ew_e (bf16, zeros at padding).
  - Messages round-trip through per-GROUP DRAM tensors md[g] so that
    aggregation for group g's blocks can start as soon as group g's
    messages are written — overlapping with message computation for
    later groups (keeps gpsimd/vector/tensor busy under the DMA wall
    and the PE warm).
  - Phase 1: per 128-edge chunk, one bf16 matmul msg = xsT_chunk^T @ W_r
    (PSUM f32), cast-copied (vector/scalar alternate) into a staging
    tile of MDG chunks, then one DMA writes the message rows to md[g].
    A host-side permutation maps xsT columns to md rows so each SBUF
    partition supplies 16 consecutive md rows -> 4KB write descriptors.
    xsT loads go on the scalar HWDGE ring, md writes on the sync ring,
    so loads and stores overlap.
  - Phase 2 (one 128-target block per iteration): one indirect gather
    pulls SL consecutive md rows for each of IV interval starts covering
    the block's per-relation runs; one tensor_tensor(is_equal) against
    an iota constant builds the 0/1 scatter matrix T (layout p (e s) so
    all operands have packed last dims -> DVE 2x mode eligible); the
    self-loop matmul plus SL accumulating matmuls out^T += mg_j^T @ T_j
    run in PSUM; the block is cast to bf16 and written out.
  - Host reassembles out (f32) from the per-core outT (bf16) blocks.
"""

import numpy as np
import ml_dtypes

import concourse.bass as bass
import concourse.bacc as bacc
import concourse.tile as tile
import concourse.mybir as mybir
from concourse.bass_utils import run_bass_kernel_spmd

F32 = mybir.dt.float32
BF16 = mybir.dt.bfloat16
I32 = mybir.dt.int32

NCORE = 8
R = 32           # num_relations (relation id R is the self-loop row of rbw)
G = 3            # target-block groups (md pipelining granularity)
GFRAC = (0.0, 0.41, 0.82, 1.0)   # group boundaries as fractions of nblk
LDTOK = 8192     # tokens per xsT load slice
MDG = 32         # chunks per md write group
PS4 = 4          # matmul outputs packed per PSUM bank (one wide copy each)
WPG = 2048       # md rows per write-slice granule
SL_CANDIDATES = (8, 10, 12, 16)
QUOTA = 7        # aggregation blocks interleaved per md write slot
LOOKAHEAD = 2    # gathers kept in flight ahead of their matmuls
BF = ml_dtypes.bfloat16


def _splits(total, step):
    out = []
    off = 0
    while off < total:
        sz = min(step, total - off)
        out.append((off, sz))
        off += sz
    return out


def host_prep(x, node_keep_mask, source, target, edge_type, edge_weights,
              bases, relation_base_weights):
    n, d = x.shape
    assert n % NCORE == 0
    nl = n // NCORE
    nblk = (nl + 127) // 128
    nlp = nblk * 128

    f32 = np.float32
    W = np.einsum("rb,bdo->rdo", relation_base_weights.astype(f32),
                  bases.astype(f32)).astype(f32)  # (R+1, 128, 128)
    wsb_h = np.ascontiguousarray(
        W.transpose(1, 0, 2).reshape(d, (R + 1) * d)).astype(BF)

    src2 = np.concatenate([source, target]).astype(np.int64)
    tgt2 = np.concatenate([target, source]).astype(np.int64)
    et2 = np.concatenate([edge_type, edge_type]).astype(np.int64)
    ew2 = np.concatenate([edge_weights, edge_weights]).astype(f32)

    owner = tgt2 // nl
    tloc = tgt2 - owner * nl
    blk = tloc // 128
    tin = (tloc - blk * 128).astype(f32)

    # block-group boundaries (G groups of consecutive blocks)
    gb = np.round(np.array(GFRAC) * nblk).astype(np.int64)
    grp = np.searchsorted(gb, blk, side="right") - 1

    # per-(core, group, rel) counts; per-(group, rel) run size uniform
    # across cores, rounded to 256 (chunk purity + the 2-row write perm
    # requires 256-row granules to stay within one relation region).
    cgr = (owner * G + grp) * R + et2
    cnt = np.bincount(cgr, minlength=NCORE * G * R).reshape(NCORE, G, R)
    Gr = ((cnt.max(axis=0) + 255) // 256) * 256          # (G, R)
    glen = ((Gr.sum(axis=1) + WPG - 1) // WPG) * WPG      # (G,) md rows/group
    gxoff = np.concatenate([[0], np.cumsum(glen)[:-1]])   # xsT col base/group
    ep1 = int(glen.sum())

    # group-local relation region starts
    starts_gr = np.cumsum(Gr, axis=1) - Gr                # (G, R) local rows

    # per-group chunk -> relation schedule (tail chunks: rel 0, zero data)
    rel_of_chunk = []
    for g in range(G):
        rc = np.repeat(np.arange(R), Gr[g] // 128)
        rc = np.concatenate([rc, np.zeros(glen[g] // 128 - len(rc),
                                          np.int64)])
        rel_of_chunk.append(tuple(int(r) for r in rc))

    # rank within (core, group, rel), ordered by target block
    starts_cgr = np.concatenate([[0], np.cumsum(cnt.reshape(-1))[:-1]])
    order = np.lexsort((blk, cgr))
    rank = np.empty(len(cgr), np.int64)
    rank[order] = np.arange(len(cgr)) - starts_cgr[cgr[order]]
    # group-local md row of each edge
    lpos = starts_gr[grp, et2] + rank

    # xsT column of md row: within each 256-row granule, partition p
    # holds rows 2p, 2p+1; row 2p+j comes from chunk j of the pair ->
    # 512B per-partition write descriptors.
    def lpos_to_xst(g, m):
        q = m // 256
        w = m % 256
        return gxoff[g] + q * 256 + (w % 2) * 128 + w // 2

    # per-(core, rel, blk) run lengths and starts within (group, rel)
    crb = (owner * R + et2) * nblk + blk
    cnt_crb = np.bincount(crb, minlength=NCORE * R * nblk).reshape(
        NCORE, R, nblk)
    # run start of block b within its (group, rel) region: cumsum over
    # blocks inside the group
    run_start = np.zeros_like(cnt_crb)
    cs = np.cumsum(cnt_crb, axis=2)
    for g in range(G):
        b0, b1 = gb[g], gb[g + 1]
        base = cs[:, :, b0 - 1][:, :, None] if b0 > 0 else 0
        run_start[:, :, b0:b1] = cs[:, :, b0:b1] - cnt_crb[:, :, b0:b1] - base

    # smallest interval stride whose per-(core, block) interval count
    # fits in <= 128 indirect-gather indices
    for SL in SL_CANDIDATES:
        n_iv = int(np.ceil(cnt_crb / SL).sum(axis=1).max())
        if n_iv <= 128:
            break
    else:
        raise AssertionError(f"no SL fits: {n_iv} intervals")
    IV = n_iv
    # per-block interval budget (max over cores) — the program gathers
    # only ivb[b] intervals for block b
    ivb = tuple(int(v) for v in
                np.ceil(cnt_crb / SL).astype(np.int64).sum(axis=1).max(axis=0))

    xf = x.astype(f32)
    keep = node_keep_mask.astype(f32)

    per_core = []
    for c in range(NCORE):
        m = owner == c
        vals = xf[src2[m]] * ew2[m, None]
        xsT_h = np.zeros((128, ep1), BF)
        xsT_h[:, lpos_to_xst(grp[m], lpos[m])] = vals.T.astype(BF)

        # group-local md row -> tin of the edge it holds (-1 if padding)
        row_tin = [np.full(int(glen[g]), -1.0, f32) for g in range(G)]
        for g in range(G):
            mg_ = m & (grp == g)
            row_tin[g][lpos[mg_]] = tin[mg_]

        # phase-2 cover: per block, interval starts covering the runs
        cidx_h = np.zeros((128, nblk), np.int32)
        tcol_h = np.full((128, nblk, SL), -1.0, f32)
        for g in range(G):
            for b in range(int(gb[g]), int(gb[g + 1])):
                p = 0
                for r in range(R):
                    s = int(starts_gr[g, r] + run_start[c, r, b])
                    ln = int(cnt_crb[c, r, b])
                    for off in range(0, ln, SL):
                        st = min(s + off, int(glen[g]) - SL)
                        lo = s + off
                        hi = min(s + off + SL, s + ln)
                        assert p < IV, "cover overflow"
                        cidx_h[p, b] = st
                        sl_rows = np.arange(st, st + SL)
                        use = (sl_rows >= lo) & (sl_rows < hi)
                        tcol_h[p, b, use] = row_tin[g][sl_rows[use]]
                        p += 1
        tcol_h = np.ascontiguousarray(
            tcol_h.reshape(128, nblk * SL)).astype(BF)

        xm = xf[c * nl:(c + 1) * nl] * keep[c * nl:(c + 1) * nl, None]
        xmt_h = np.zeros((128, nlp), BF)
        xmt_h[:, :nl] = xm.T.astype(BF)

        per_core.append({
            "xsT": xsT_h,
            "wsb": wsb_h,
            "xmt": xmt_h,
            "cidx": np.ascontiguousarray(cidx_h),
            "tcol": tcol_h,
        })

    cfg = dict(n=n, nl=nl, nblk=nblk, nlp=nlp, ep1=ep1, SL=SL, IV=IV,
               ivb=ivb,
               gb=tuple(int(v) for v in gb),
               glen=tuple(int(v) for v in glen),
               rel_of_chunk=tuple(rel_of_chunk))
    return per_core, cfg


def build_program(cfg):
    nblk = cfg["nblk"]
    nlp = cfg["nlp"]
    ep1 = cfg["ep1"]
    SL = cfg["SL"]
    ivb = cfg["ivb"]
    gb = cfg["gb"]
    glen = cfg["glen"]
    rel_of_chunk = cfg["rel_of_chunk"]

    nc = bacc.Bacc(None, target_bir_lowering=False, debug=False)

    xsT = nc.declare_dram_parameter("xsT", [128, ep1], BF16, isOutput=False)
    wsb = nc.declare_dram_parameter("wsb", [128, (R + 1) * 128], BF16,
                                    isOutput=False)
    xmt = nc.declare_dram_parameter("xmt", [128, nlp], BF16, isOutput=False)
    cidx = nc.declare_dram_parameter("cidx", [128, nblk], I32, isOutput=False)
    tcol = nc.declare_dram_parameter("tcol", [128, nblk * SL], BF16,
                                     isOutput=False)
    outT = nc.declare_dram_parameter("outT", [128, nlp], BF16, isOutput=True)

    md = [nc.dram_tensor(f"md{g}", [glen[g], 128], BF16) for g in range(G)]

    # colw[p, s*128 + e] = e  (iota over e, tiled SL times)
    colw_d = nc.inline_tensor(
        np.tile(np.arange(128, dtype=np.float32), (128, SL)),
        name="colw_c")

    gxoff = np.concatenate([[0], np.cumsum(glen)[:-1]]).astype(np.int64)

    with tile.TileContext(nc) as tc:
        with tc.tile_pool(name="const", bufs=1) as constp:
            wsb_t = constp.tile([128, (R + 1) * 128], BF16)
            nc.sync.dma_start(out=wsb_t[:], in_=wsb[:])
            xmt_t = constp.tile([128, nlp], BF16)
            nc.sync.dma_start(out=xmt_t[:], in_=xmt[:])
            cidx_t = constp.tile([128, nblk], I32)
            nc.sync.dma_start(out=cidx_t[:], in_=cidx[:])
            tcol_t = constp.tile([128, nblk * SL], BF16)
            nc.sync.dma_start(out=tcol_t[:], in_=tcol[:])
            colw_f = constp.tile([128, SL * 128], F32)
            nc.sync.dma_start(out=colw_f[:], in_=colw_d[:])
            colw = constp.tile([128, SL * 128], BF16)
            nc.vector.tensor_copy(out=colw[:], in_=colw_f[:])

            with (
                tc.tile_pool(name="xt", bufs=3) as xp,
                tc.tile_pool(name="mds", bufs=3) as mdp,
                tc.tile_pool(name="p1ps", bufs=5, space="PSUM") as p1ps,
                tc.tile_pool(name="p2", bufs=10) as p2,
                tc.tile_pool(name="p2t", bufs=10) as p2t,
                tc.tile_pool(name="io", bufs=2) as iop,
                tc.tile_pool(name="p2ps", bufs=3, space="PSUM") as p2ps,
            ):
                state = {"ncopy": 0, "obt": None, "obn": 0, "obb": -1}
                ready = []    # blocks whose md group is complete

                def flush_ob():
                    # one DMA for the accumulated run of consecutive blocks
                    if state["obn"]:
                        b0 = state["obb"] - state["obn"] + 1
                        nc.scalar.dma_start(
                            out=outT[:, 128 * b0:128 * (b0 + state["obn"])],
                            in_=state["obt"][:, :128 * state["obn"]])
                    state["obt"] = None
                    state["obn"] = 0

                def emit_block(b, g, iv):
                    mg = p2.tile([128, SL * 128], BF16, tag="mg")
                    nc.gpsimd.indirect_dma_start(
                        out=mg[0:iv, :], out_offset=None, in_=md[g][:, :],
                        in_offset=bass.IndirectOffsetOnAxis(
                            ap=cidx_t[0:iv, b:b + 1], axis=0))
                    tt = p2t.tile([128, SL * 128], BF16, tag="T")
                    tsl = tcol_t[0:iv, b * SL:(b + 1) * SL]
                    tb = tsl.unsqueeze(2).broadcast_to([iv, SL, 128])
                    nc.vector.tensor_tensor(
                        out=tt[0:iv].rearrange("p (s e) -> p s e", e=128),
                        in0=colw[0:iv].rearrange("p (s e) -> p s e", e=128),
                        in1=tb, op=mybir.AluOpType.is_equal)
                    ps = p2ps.tile([128, 128], F32, tag="acc")
                    nc.tensor.matmul(
                        out=ps[:],
                        lhsT=wsb_t[:, R * 128:(R + 1) * 128],
                        rhs=xmt_t[:, 128 * b:128 * (b + 1)],
                        start=True, stop=False)
                    for j in range(SL):
                        nc.tensor.matmul(
                            out=ps[:],
                            lhsT=mg[0:iv, 128 * j:128 * (j + 1)],
                            rhs=tt[0:iv, 128 * j:128 * (j + 1)],
                            start=False, stop=(j == SL - 1))
                    if state["obn"] and b != state["obb"] + 1:
                        flush_ob()
                    if state["obt"] is None:
                        obt = iop.tile([128, 8 * 128], BF16, tag="ob")
                        state["obt"] = obt
                    k = state["obn"]
                    nc.scalar.copy(out=state["obt"][:, 128 * k:128 * (k + 1)],
                                   in_=ps[:])
                    state["obb"] = b
                    state["obn"] = k + 1
                    if state["obn"] == 8:
                        flush_ob()

                def drain(k):
                    nn = 0
                    while ready and nn < k:
                        emit_block(*ready.pop(0))
                        nn += 1
                    flush_ob()

                # ---------------- Phase 1: messages ----------------
                for g in range(G):
                    roc = rel_of_chunk[g]
                    for loff, lsz in _splits(glen[g], LDTOK):
                        xt = xp.tile([128, LDTOK], BF16, tag="xt")
                        nc.scalar.dma_start(
                            out=xt[:, :lsz],
                            in_=xsT[:, gxoff[g] + loff:gxoff[g] + loff + lsz])
                        for goff, gsz in _splits(lsz, MDG * 128):
                            ms = mdp.tile([128, MDG * 128], BF16, tag="ms")
                            for poff, psz in _splits(gsz, PS4 * 128):
                                ps = p1ps.tile([128, PS4 * 128], F32,
                                               tag="ps")
                                for j in range(psz // 128):
                                    ch = (loff + goff + poff) // 128 + j
                                    r = roc[ch]
                                    co = goff + poff + 128 * j
                                    nc.tensor.matmul(
                                        out=ps[:, 128 * j:128 * (j + 1)],
                                        lhsT=xt[:, co:co + 128],
                                        rhs=wsb_t[:, 128 * r:128 * (r + 1)],
                                        start=True, stop=True)
                                state["ncopy"] += 1
                                if state["ncopy"] % 3 == 0:
                                    nc.vector.tensor_copy(
                                        out=ms[:, poff:poff + psz],
                                        in_=ps[:, :psz])
                                else:
                                    nc.scalar.copy(
                                        out=ms[:, poff:poff + psz],
                                        in_=ps[:, :psz])
                            r0 = loff + goff
                            mdv = md[g][r0:r0 + gsz, :]
                            nc.sync.dma_start(
                                out=mdv.rearrange("(w p j) c -> p w (j c)",
                                                  p=128, j=2),
                                in_=ms[:, :gsz].rearrange("p (w k) -> p w k",
                                                          k=256))
                            drain(QUOTA)
                    # group g's messages fully emitted -> its blocks ready
                    ready.extend((b, g, ivb[b])
                                 for b in range(gb[g], gb[g + 1]))

                # ---------------- Phase 2 tail ----------------
                drain(len(ready))
                flush_ob()

    nc.finalize()
    return nc


_PROGRAM_CACHE = {}


def _get_program(cfg):
    key = tuple(sorted((k, str(v)) for k, v in cfg.items()))
    if key not in _PROGRAM_CACHE:
        _PROGRAM_CACHE[key] = build_program(cfg)
    return _PROGRAM_CACHE[key]


def kernel(x, node_keep_mask, source, target, edge_type, edge_weights,
           bases, relation_base_weights):
    per_core, cfg = host_prep(x, node_keep_mask, source, target, edge_type,
                              edge_weights, bases, relation_base_weights)
    nc = _get_program(cfg)
    res = run_bass_kernel_spmd(nc, per_core, list(range(NCORE)))
    nl = cfg["nl"]
    out = np.empty((cfg["n"], 128), np.float32)
    for c in range(NCORE):
        out[c * nl:(c + 1) * nl] = \
            res.results[c]["outT"][:, :nl].T.astype(np.float32)
    return out


# revision 39
# speedup vs baseline: 1.0604x; 1.0569x over previous
"""BasesDecomposition (R-GCN style) message passing kernel for Trainium2.

Strategy (8 NeuronCores, SPMD — one program, per-core data):
  - Nodes sharded by row: core c owns targets [c*NL, (c+1)*NL).
  - Edges symmetrized on host, partitioned by target-owner core, then by
    target BLOCK-GROUP (G groups of ~25 blocks), then by relation (each
    (group, rel) run padded to a cross-core-uniform multiple of 128 so
    the chunk -> W_r schedule is program-static); within a (group, rel)
    run, edges are ordered by target block.
  - Per-relation weights W_r = sum_b rbw[r, b] * bases[b] (host, bf16).
  - Host builds, per core, a pre-transposed pre-scaled source table
    xsT[d, pos] = x[src_e, d] * ew_e (bf16, zeros at padding).
  - Messages round-trip through per-GROUP DRAM tensors md[g] so that
    aggregation for group g's blocks can start as soon as group g's
    messages are written — overlapping with message computation for
    later groups (keeps gpsimd/vector/tensor busy under the DMA wall
    and the PE warm).
  - Phase 1: per 128-edge chunk, one bf16 matmul msg = xsT_chunk^T @ W_r
    (PSUM f32), cast-copied (vector/scalar alternate) into a staging
    tile of MDG chunks, then one DMA writes the message rows to md[g].
    A host-side permutation maps xsT columns to md rows so each SBUF
    partition supplies 16 consecutive md rows -> 4KB write descriptors.
    xsT loads go on the scalar HWDGE ring, md writes on the sync ring,
    so loads and stores overlap.
  - Phase 2 (one 128-target block per iteration): one indirect gather
    pulls SL consecutive md rows for each of IV interval starts covering
    the block's per-relation runs; one tensor_tensor(is_equal) against
    an iota constant builds the 0/1 scatter matrix T (layout p (e s) so
    all operands have packed last dims -> DVE 2x mode eligible); the
    self-loop matmul plus SL accumulating matmuls out^T += mg_j^T @ T_j
    run in PSUM; the block is cast to bf16 and written out.
  - Host reassembles out (f32) from the per-core outT (bf16) blocks.
"""

import numpy as np
import ml_dtypes

import concourse.bass as bass
import concourse.bacc as bacc
import concourse.tile as tile
import concourse.mybir as mybir
from concourse.bass_utils import run_bass_kernel_spmd

F32 = mybir.dt.float32
BF16 = mybir.dt.bfloat16
I32 = mybir.dt.int32

NCORE = 8
R = 32           # num_relations (relation id R is the self-loop row of rbw)
G = 3            # target-block groups (md pipelining granularity)
GFRAC = (0.0, 0.41, 0.82, 1.0)   # group boundaries as fractions of nblk
LDTOK = 8192     # tokens per xsT load slice
MDG = 32         # chunks per md write group
PS4 = 4          # matmul outputs packed per PSUM bank (one wide copy each)
WPG = 2048       # md rows per write-slice granule
SL_CANDIDATES = (8, 10, 12, 16)
QUOTA = 7        # aggregation blocks interleaved per md write slot
LOOKAHEAD = 2    # gathers kept in flight ahead of their matmuls
BF = ml_dtypes.bfloat16


def _splits(total, step):
    out = []
    off = 0
    while off < total:
        sz = min(step, total - off)
        out.append((off, sz))
        off += sz
    return out


def host_prep(x, node_keep_mask, source, target, edge_type, edge_weights,
              bases, relation_base_weights):
    n, d = x.shape
    assert n % NCORE == 0
    nl = n // NCORE
    nblk = (nl + 127) // 128
    nlp = nblk * 128

    f32 = np.float32
    W = np.einsum("rb,bdo->rdo", relation_base_weights.astype(f32),
                  bases.astype(f32)).astype(f32)  # (R+1, 128, 128)
    wsb_h = np.ascontiguousarray(
        W.transpose(1, 0, 2).reshape(d, (R + 1) * d)).astype(BF)

    src2 = np.concatenate([source, target]).astype(np.int64)
    tgt2 = np.concatenate([target, source]).astype(np.int64)
    et2 = np.concatenate([edge_type, edge_type]).astype(np.int64)
    ew2 = np.concatenate([edge_weights, edge_weights]).astype(f32)

    owner = tgt2 // nl
    tloc = tgt2 - owner * nl
    blk = tloc // 128
    tin = (tloc - blk * 128).astype(f32)

    # block-group boundaries (G groups of consecutive blocks)
    gb = np.round(np.array(GFRAC) * nblk).astype(np.int64)
    grp = np.searchsorted(gb, blk, side="right") - 1

    # per-(core, group, rel) counts; per-(group, rel) run size uniform
    # across cores, rounded to 256 (chunk purity + the 2-row write perm
    # requires 256-row granules to stay within one relation region).
    cgr = (owner * G + grp) * R + et2
    cnt = np.bincount(cgr, minlength=NCORE * G * R).reshape(NCORE, G, R)
    Gr = ((cnt.max(axis=0) + 255) // 256) * 256          # (G, R)
    glen = ((Gr.sum(axis=1) + WPG - 1) // WPG) * WPG      # (G,) md rows/group
    gxoff = np.concatenate([[0], np.cumsum(glen)[:-1]])   # xsT col base/group
    ep1 = int(glen.sum())

    # group-local relation region starts
    starts_gr = np.cumsum(Gr, axis=1) - Gr                # (G, R) local rows

    # per-group chunk -> relation schedule (tail chunks: rel 0, zero data)
    rel_of_chunk = []
    for g in range(G):
        rc = np.repeat(np.arange(R), Gr[g] // 128)
        rc = np.concatenate([rc, np.zeros(glen[g] // 128 - len(rc),
                                          np.int64)])
        rel_of_chunk.append(tuple(int(r) for r in rc))

    # rank within (core, group, rel), ordered by target block
    starts_cgr = np.concatenate([[0], np.cumsum(cnt.reshape(-1))[:-1]])
    order = np.lexsort((blk, cgr))
    rank = np.empty(len(cgr), np.int64)
    rank[order] = np.arange(len(cgr)) - starts_cgr[cgr[order]]
    # group-local md row of each edge
    lpos = starts_gr[grp, et2] + rank

    # xsT column of md row: within each 256-row granule, partition p
    # holds rows 2p, 2p+1; row 2p+j comes from chunk j of the pair ->
    # 512B per-partition write descriptors.
    def lpos_to_xst(g, m):
        q = m // 256
        w = m % 256
        return gxoff[g] + q * 256 + (w % 2) * 128 + w // 2

    # per-(core, rel, blk) run lengths and starts within (group, rel)
    crb = (owner * R + et2) * nblk + blk
    cnt_crb = np.bincount(crb, minlength=NCORE * R * nblk).reshape(
        NCORE, R, nblk)
    # run start of block b within its (group, rel) region: cumsum over
    # blocks inside the group
    run_start = np.zeros_like(cnt_crb)
    cs = np.cumsum(cnt_crb, axis=2)
    for g in range(G):
        b0, b1 = gb[g], gb[g + 1]
        base = cs[:, :, b0 - 1][:, :, None] if b0 > 0 else 0
        run_start[:, :, b0:b1] = cs[:, :, b0:b1] - cnt_crb[:, :, b0:b1] - base

    # smallest interval stride whose per-(core, block) interval count
    # fits in <= 128 indirect-gather indices
    for SL in SL_CANDIDATES:
        n_iv = int(np.ceil(cnt_crb / SL).sum(axis=1).max())
        if n_iv <= 128:
            break
    else:
        raise AssertionError(f"no SL fits: {n_iv} intervals")
    IV = n_iv
    # per-block interval budget (max over cores) — the program gathers
    # only ivb[b] intervals for block b
    ivb = tuple(int(v) for v in
                np.ceil(cnt_crb / SL).astype(np.int64).sum(axis=1).max(axis=0))

    xf = x.astype(f32)
    keep = node_keep_mask.astype(f32)

    per_core = []
    for c in range(NCORE):
        m = owner == c
        vals = xf[src2[m]] * ew2[m, None]
        xsT_h = np.zeros((128, ep1), BF)
        xsT_h[:, lpos_to_xst(grp[m], lpos[m])] = vals.T.astype(BF)

        # group-local md row -> tin of the edge it holds (-1 if padding)
        row_tin = [np.full(int(glen[g]), -1.0, f32) for g in range(G)]
        for g in range(G):
            mg_ = m & (grp == g)
            row_tin[g][lpos[mg_]] = tin[mg_]

        # phase-2 cover: per block, interval starts covering the runs
        cidx_h = np.zeros((128, nblk), np.int32)
        tcol_h = np.full((128, nblk, SL), -1.0, f32)
        for g in range(G):
            for b in range(int(gb[g]), int(gb[g + 1])):
                p = 0
                for r in range(R):
                    s = int(starts_gr[g, r] + run_start[c, r, b])
                    ln = int(cnt_crb[c, r, b])
                    for off in range(0, ln, SL):
                        st = min(s + off, int(glen[g]) - SL)
                        lo = s + off
                        hi = min(s + off + SL, s + ln)
                        assert p < IV, "cover overflow"
                        cidx_h[p, b] = st
                        sl_rows = np.arange(st, st + SL)
                        use = (sl_rows >= lo) & (sl_rows < hi)
                        tcol_h[p, b, use] = row_tin[g][sl_rows[use]]
                        p += 1
        tcol_h = np.ascontiguousarray(
            tcol_h.reshape(128, nblk * SL)).astype(BF)

        xm = xf[c * nl:(c + 1) * nl] * keep[c * nl:(c + 1) * nl, None]
        xmt_h = np.zeros((128, nlp), BF)
        xmt_h[:, :nl] = xm.T.astype(BF)

        per_core.append({
            "xsT": xsT_h,
            "wsb": wsb_h,
            "xmt": xmt_h,
            "cidx": np.ascontiguousarray(cidx_h),
            "tcol": tcol_h,
        })

    cfg = dict(n=n, nl=nl, nblk=nblk, nlp=nlp, ep1=ep1, SL=SL, IV=IV,
               ivb=ivb,
               gb=tuple(int(v) for v in gb),
               glen=tuple(int(v) for v in glen),
               rel_of_chunk=tuple(rel_of_chunk))
    return per_core, cfg


def build_program(cfg):
    nblk = cfg["nblk"]
    nlp = cfg["nlp"]
    ep1 = cfg["ep1"]
    SL = cfg["SL"]
    IV = cfg["IV"]
    gb = cfg["gb"]
    glen = cfg["glen"]
    rel_of_chunk = cfg["rel_of_chunk"]

    nc = bacc.Bacc(None, target_bir_lowering=False, debug=False)

    xsT = nc.declare_dram_parameter("xsT", [128, ep1], BF16, isOutput=False)
    wsb = nc.declare_dram_parameter("wsb", [128, (R + 1) * 128], BF16,
                                    isOutput=False)
    xmt = nc.declare_dram_parameter("xmt", [128, nlp], BF16, isOutput=False)
    cidx = nc.declare_dram_parameter("cidx", [128, nblk], I32, isOutput=False)
    tcol = nc.declare_dram_parameter("tcol", [128, nblk * SL], BF16,
                                     isOutput=False)
    outT = nc.declare_dram_parameter("outT", [128, nlp], BF16, isOutput=True)

    md = [nc.dram_tensor(f"md{g}", [glen[g], 128], BF16) for g in range(G)]

    # colw[p, s*128 + e] = e  (iota over e, tiled SL times)
    colw_d = nc.inline_tensor(
        np.tile(np.arange(128, dtype=np.float32), (128, SL)),
        name="colw_c")

    gxoff = np.concatenate([[0], np.cumsum(glen)[:-1]]).astype(np.int64)

    with tile.TileContext(nc) as tc:
        with tc.tile_pool(name="const", bufs=1) as constp:
            wsb_t = constp.tile([128, (R + 1) * 128], BF16)
            nc.sync.dma_start(out=wsb_t[:], in_=wsb[:])
            xmt_t = constp.tile([128, nlp], BF16)
            nc.sync.dma_start(out=xmt_t[:], in_=xmt[:])
            cidx_t = constp.tile([128, nblk], I32)
            nc.sync.dma_start(out=cidx_t[:], in_=cidx[:])
            tcol_t = constp.tile([128, nblk * SL], BF16)
            nc.sync.dma_start(out=tcol_t[:], in_=tcol[:])
            colw_f = constp.tile([128, SL * 128], F32)
            nc.sync.dma_start(out=colw_f[:], in_=colw_d[:])
            colw = constp.tile([128, SL * 128], BF16)
            nc.vector.tensor_copy(out=colw[:], in_=colw_f[:])

            with (
                tc.tile_pool(name="xt", bufs=4) as xp,
                tc.tile_pool(name="mds", bufs=3) as mdp,
                tc.tile_pool(name="p1ps", bufs=5, space="PSUM") as p1ps,
                tc.tile_pool(name="p2", bufs=6) as p2,
                tc.tile_pool(name="p2t", bufs=6) as p2t,
                tc.tile_pool(name="io", bufs=2) as iop,
                tc.tile_pool(name="p2ps", bufs=3, space="PSUM") as p2ps,
            ):
                state = {"ncopy": 0, "obt": None, "obn": 0, "obb": -1}
                ready = []    # blocks whose md group is complete

                def flush_ob():
                    # one DMA for the accumulated run of consecutive blocks
                    if state["obn"]:
                        b0 = state["obb"] - state["obn"] + 1
                        nc.scalar.dma_start(
                            out=outT[:, 128 * b0:128 * (b0 + state["obn"])],
                            in_=state["obt"][:, :128 * state["obn"]])
                    state["obt"] = None
                    state["obn"] = 0

                def emit_block(b, g):
                    iv = IV
                    mg = p2.tile([128, SL * 128], BF16, tag="mg")
                    nc.gpsimd.indirect_dma_start(
                        out=mg[0:iv, :], out_offset=None, in_=md[g][:, :],
                        in_offset=bass.IndirectOffsetOnAxis(
                            ap=cidx_t[0:iv, b:b + 1], axis=0))
                    tt = p2t.tile([128, SL * 128], BF16, tag="T")
                    tsl = tcol_t[0:iv, b * SL:(b + 1) * SL]
                    tb = tsl.unsqueeze(2).broadcast_to([iv, SL, 128])
                    nc.vector.tensor_tensor(
                        out=tt[0:iv].rearrange("p (s e) -> p s e", e=128),
                        in0=colw[0:iv].rearrange("p (s e) -> p s e", e=128),
                        in1=tb, op=mybir.AluOpType.is_equal)
                    ps = p2ps.tile([128, 128], F32, tag="acc")
                    nc.tensor.matmul(
                        out=ps[:],
                        lhsT=wsb_t[:, R * 128:(R + 1) * 128],
                        rhs=xmt_t[:, 128 * b:128 * (b + 1)],
                        start=True, stop=False)
                    for j in range(SL):
                        nc.tensor.matmul(
                            out=ps[:],
                            lhsT=mg[0:iv, 128 * j:128 * (j + 1)],
                            rhs=tt[0:iv, 128 * j:128 * (j + 1)],
                            start=False, stop=(j == SL - 1))
                    if state["obn"] and b != state["obb"] + 1:
                        flush_ob()
                    if state["obt"] is None:
                        obt = iop.tile([128, 8 * 128], BF16, tag="ob")
                        state["obt"] = obt
                    k = state["obn"]
                    nc.scalar.copy(out=state["obt"][:, 128 * k:128 * (k + 1)],
                                   in_=ps[:])
                    state["obb"] = b
                    state["obn"] = k + 1
                    if state["obn"] == 8:
                        flush_ob()

                def drain(k):
                    nn = 0
                    while ready and nn < k:
                        emit_block(*ready.pop(0))
                        nn += 1
                    flush_ob()

                # ---------------- Phase 1: messages ----------------
                slices = [(g, loff, lsz) for g in range(G)
                          for loff, lsz in _splits(glen[g], LDTOK)]
                xts = {}

                def emit_load(i):
                    g, loff, lsz = slices[i]
                    xt = xp.tile([128, LDTOK], BF16, tag="xt")
                    nc.scalar.dma_start(
                        out=xt[:, :lsz],
                        in_=xsT[:, gxoff[g] + loff:gxoff[g] + loff + lsz])
                    xts[i] = xt

                emit_load(0)
                for i, (g, loff, lsz) in enumerate(slices):
                    roc = rel_of_chunk[g]
                    # prefetch the next slice's load ahead of this slice's
                    # compute so the scalar-ring dispatch isn't queued
                    # behind this slice's copies
                    if i + 1 < len(slices):
                        emit_load(i + 1)
                    xt = xts.pop(i)
                    if True:
                        for goff, gsz in _splits(lsz, MDG * 128):
                            ms = mdp.tile([128, MDG * 128], BF16, tag="ms")
                            for poff, psz in _splits(gsz, PS4 * 128):
                                ps = p1ps.tile([128, PS4 * 128], F32,
                                               tag="ps")
                                for j in range(psz // 128):
                                    ch = (loff + goff + poff) // 128 + j
                                    r = roc[ch]
                                    co = goff + poff + 128 * j
                                    nc.tensor.matmul(
                                        out=ps[:, 128 * j:128 * (j + 1)],
                                        lhsT=xt[:, co:co + 128],
                                        rhs=wsb_t[:, 128 * r:128 * (r + 1)],
                                        start=True, stop=True)
                                state["ncopy"] += 1
                                if state["ncopy"] % 3 == 0:
                                    nc.vector.tensor_copy(
                                        out=ms[:, poff:poff + psz],
                                        in_=ps[:, :psz])
                                else:
                                    nc.scalar.copy(
                                        out=ms[:, poff:poff + psz],
                                        in_=ps[:, :psz])
                            r0 = loff + goff
                            mdv = md[g][r0:r0 + gsz, :]
                            nc.sync.dma_start(
                                out=mdv.rearrange("(w p j) c -> p w (j c)",
                                                  p=128, j=2),
                                in_=ms[:, :gsz].rearrange("p (w k) -> p w k",
                                                          k=256))
                            drain(QUOTA)
                    # group g's messages fully emitted -> its blocks ready
                    if i + 1 == len(slices) or slices[i + 1][0] != g:
                        ready.extend((b, g)
                                     for b in range(gb[g], gb[g + 1]))

                # ---------------- Phase 2 tail ----------------
                drain(len(ready))
                flush_ob()

    nc.finalize()
    return nc


_PROGRAM_CACHE = {}


def _get_program(cfg):
    key = tuple(sorted((k, str(v)) for k, v in cfg.items()))
    if key not in _PROGRAM_CACHE:
        _PROGRAM_CACHE[key] = build_program(cfg)
    return _PROGRAM_CACHE[key]


def kernel(x, node_keep_mask, source, target, edge_type, edge_weights,
           bases, relation_base_weights):
    per_core, cfg = host_prep(x, node_keep_mask, source, target, edge_type,
                              edge_weights, bases, relation_base_weights)
    nc = _get_program(cfg)
    res = run_bass_kernel_spmd(nc, per_core, list(range(NCORE)))
    nl = cfg["nl"]
    out = np.empty((cfg["n"], 128), np.float32)
    for c in range(NCORE):
        out[c * nl:(c + 1) * nl] = \
            res.results[c]["outT"][:, :nl].T.astype(np.float32)
    return out


# revision 43
# speedup vs baseline: 1.1803x; 1.1132x over previous
"""BasesDecomposition (R-GCN style) message passing kernel for Trainium2.

Strategy (8 NeuronCores, SPMD — one program, per-core data):
  - Nodes sharded by row: core c owns targets [c*NL, (c+1)*NL).
  - Edges symmetrized on host, partitioned by target-owner core, then by
    target BLOCK-GROUP (G groups of ~25 blocks), then by relation (each
    (group, rel) run padded to a cross-core-uniform multiple of 128 so
    the chunk -> W_r schedule is program-static); within a (group, rel)
    run, edges are ordered by target block.
  - Per-relation weights W_r = sum_b rbw[r, b] * bases[b] (host, bf16).
  - Host builds, per core, a pre-transposed pre-scaled source table
    xsT[d, pos] = x[src_e, d] * ew_e (bf16, zeros at padding).
  - Messages round-trip through per-GROUP DRAM tensors md[g] so that
    aggregation for group g's blocks can start as soon as group g's
    messages are written — overlapping with message computation for
    later groups (keeps gpsimd/vector/tensor busy under the DMA wall
    and the PE warm).
  - Phase 1: per 128-edge chunk, one bf16 matmul msg = xsT_chunk^T @ W_r
    (PSUM f32), cast-copied (vector/scalar alternate) into a staging
    tile of MDG chunks, then one DMA writes the message rows to md[g].
    A host-side permutation maps xsT columns to md rows so each SBUF
    partition supplies 16 consecutive md rows -> 4KB write descriptors.
    xsT loads go on the scalar HWDGE ring, md writes on the sync ring,
    so loads and stores overlap.
  - Phase 2 (one 128-target block per iteration): one indirect gather
    pulls SL consecutive md rows for each of IV interval starts covering
    the block's per-relation runs; one tensor_tensor(is_equal) against
    an iota constant builds the 0/1 scatter matrix T (layout p (e s) so
    all operands have packed last dims -> DVE 2x mode eligible); the
    self-loop matmul plus SL accumulating matmuls out^T += mg_j^T @ T_j
    run in PSUM; the block is cast to bf16 and written out.
  - Host reassembles out (f32) from the per-core outT (bf16) blocks.
"""

import numpy as np
import ml_dtypes

import concourse.bass as bass
import concourse.bacc as bacc
import concourse.tile as tile
import concourse.mybir as mybir
from concourse.bass_utils import run_bass_kernel_spmd

F32 = mybir.dt.float32
BF16 = mybir.dt.bfloat16
I32 = mybir.dt.int32

NCORE = 8
R = 32           # num_relations (relation id R is the self-loop row of rbw)
G = 3            # target-block groups (md pipelining granularity)
GFRAC = (0.0, 0.41, 0.82, 1.0)   # group boundaries as fractions of nblk
LDTOK = 8192     # tokens per xsT load slice
MDG = 32         # chunks per md write group
PS4 = 4          # matmul outputs packed per PSUM bank (one wide copy each)
WPG = 2048       # md rows per write-slice granule
SL_CANDIDATES = (8, 10, 12, 16)
QUOTA = 7        # aggregation blocks interleaved per md write slot
SEED = 4         # gathers pre-emitted at each group boundary
BF = ml_dtypes.bfloat16


def _splits(total, step):
    out = []
    off = 0
    while off < total:
        sz = min(step, total - off)
        out.append((off, sz))
        off += sz
    return out


def host_prep(x, node_keep_mask, source, target, edge_type, edge_weights,
              bases, relation_base_weights):
    n, d = x.shape
    assert n % NCORE == 0
    nl = n // NCORE
    nblk = (nl + 127) // 128
    nlp = nblk * 128

    f32 = np.float32
    W = np.einsum("rb,bdo->rdo", relation_base_weights.astype(f32),
                  bases.astype(f32)).astype(f32)  # (R+1, 128, 128)
    wsb_h = np.ascontiguousarray(
        W.transpose(1, 0, 2).reshape(d, (R + 1) * d)).astype(BF)

    src2 = np.concatenate([source, target]).astype(np.int64)
    tgt2 = np.concatenate([target, source]).astype(np.int64)
    et2 = np.concatenate([edge_type, edge_type]).astype(np.int64)
    ew2 = np.concatenate([edge_weights, edge_weights]).astype(f32)

    owner = tgt2 // nl
    tloc = tgt2 - owner * nl
    blk = tloc // 128
    tin = (tloc - blk * 128).astype(f32)

    # block-group boundaries (G groups of consecutive blocks)
    gb = np.round(np.array(GFRAC) * nblk).astype(np.int64)
    grp = np.searchsorted(gb, blk, side="right") - 1

    # per-(core, group, rel) counts; per-(group, rel) run size uniform
    # across cores, rounded to 256 (chunk purity + the 2-row write perm
    # requires 256-row granules to stay within one relation region).
    cgr = (owner * G + grp) * R + et2
    cnt = np.bincount(cgr, minlength=NCORE * G * R).reshape(NCORE, G, R)
    Gr = ((cnt.max(axis=0) + 255) // 256) * 256          # (G, R)
    glen = ((Gr.sum(axis=1) + WPG - 1) // WPG) * WPG      # (G,) md rows/group
    gxoff = np.concatenate([[0], np.cumsum(glen)[:-1]])   # xsT col base/group
    ep1 = int(glen.sum())

    # group-local relation region starts
    starts_gr = np.cumsum(Gr, axis=1) - Gr                # (G, R) local rows

    # per-group chunk -> relation schedule (tail chunks: rel 0, zero data)
    rel_of_chunk = []
    for g in range(G):
        rc = np.repeat(np.arange(R), Gr[g] // 128)
        rc = np.concatenate([rc, np.zeros(glen[g] // 128 - len(rc),
                                          np.int64)])
        rel_of_chunk.append(tuple(int(r) for r in rc))

    # rank within (core, group, rel), ordered by target block
    starts_cgr = np.concatenate([[0], np.cumsum(cnt.reshape(-1))[:-1]])
    order = np.lexsort((blk, cgr))
    rank = np.empty(len(cgr), np.int64)
    rank[order] = np.arange(len(cgr)) - starts_cgr[cgr[order]]
    # group-local md row of each edge
    lpos = starts_gr[grp, et2] + rank

    # xsT column of md row: within each 256-row granule, partition p
    # holds rows 2p, 2p+1; row 2p+j comes from chunk j of the pair ->
    # 512B per-partition write descriptors.
    def lpos_to_xst(g, m):
        q = m // 256
        w = m % 256
        return gxoff[g] + q * 256 + (w % 2) * 128 + w // 2

    # per-(core, rel, blk) run lengths and starts within (group, rel)
    crb = (owner * R + et2) * nblk + blk
    cnt_crb = np.bincount(crb, minlength=NCORE * R * nblk).reshape(
        NCORE, R, nblk)
    # run start of block b within its (group, rel) region: cumsum over
    # blocks inside the group
    run_start = np.zeros_like(cnt_crb)
    cs = np.cumsum(cnt_crb, axis=2)
    for g in range(G):
        b0, b1 = gb[g], gb[g + 1]
        base = cs[:, :, b0 - 1][:, :, None] if b0 > 0 else 0
        run_start[:, :, b0:b1] = cs[:, :, b0:b1] - cnt_crb[:, :, b0:b1] - base

    # smallest interval stride whose per-(core, block) interval count
    # fits in <= 128 indirect-gather indices
    for SL in SL_CANDIDATES:
        n_iv = int(np.ceil(cnt_crb / SL).sum(axis=1).max())
        if n_iv <= 128:
            break
    else:
        raise AssertionError(f"no SL fits: {n_iv} intervals")
    IV = n_iv
    # per-block interval budget (max over cores) — the program gathers
    # only ivb[b] intervals for block b
    ivb = tuple(int(v) for v in
                np.ceil(cnt_crb / SL).astype(np.int64).sum(axis=1).max(axis=0))

    xf = x.astype(f32)
    keep = node_keep_mask.astype(f32)

    per_core = []
    for c in range(NCORE):
        m = owner == c
        vals = xf[src2[m]] * ew2[m, None]
        xsT_h = np.zeros((128, ep1), BF)
        xsT_h[:, lpos_to_xst(grp[m], lpos[m])] = vals.T.astype(BF)

        # group-local md row -> tin of the edge it holds (-1 if padding)
        row_tin = [np.full(int(glen[g]), -1.0, f32) for g in range(G)]
        for g in range(G):
            mg_ = m & (grp == g)
            row_tin[g][lpos[mg_]] = tin[mg_]

        # phase-2 cover: per block, interval starts covering the runs
        cidx_h = np.zeros((128, nblk), np.int32)
        tcol_h = np.full((128, nblk, SL), -1.0, f32)
        for g in range(G):
            for b in range(int(gb[g]), int(gb[g + 1])):
                p = 0
                for r in range(R):
                    s = int(starts_gr[g, r] + run_start[c, r, b])
                    ln = int(cnt_crb[c, r, b])
                    for off in range(0, ln, SL):
                        st = min(s + off, int(glen[g]) - SL)
                        lo = s + off
                        hi = min(s + off + SL, s + ln)
                        assert p < IV, "cover overflow"
                        cidx_h[p, b] = st
                        sl_rows = np.arange(st, st + SL)
                        use = (sl_rows >= lo) & (sl_rows < hi)
                        tcol_h[p, b, use] = row_tin[g][sl_rows[use]]
                        p += 1
        tcol_h = np.ascontiguousarray(
            tcol_h.reshape(128, nblk * SL)).astype(BF)

        xm = xf[c * nl:(c + 1) * nl] * keep[c * nl:(c + 1) * nl, None]
        xmt_h = np.zeros((128, nlp), BF)
        xmt_h[:, :nl] = xm.T.astype(BF)

        per_core.append({
            "xsT": xsT_h,
            "wsb": wsb_h,
            "xmt": xmt_h,
            "cidx": np.ascontiguousarray(cidx_h),
            "tcol": tcol_h,
        })

    cfg = dict(n=n, nl=nl, nblk=nblk, nlp=nlp, ep1=ep1, SL=SL, IV=IV,
               ivb=ivb,
               gb=tuple(int(v) for v in gb),
               glen=tuple(int(v) for v in glen),
               rel_of_chunk=tuple(rel_of_chunk))
    return per_core, cfg


def build_program(cfg):
    nblk = cfg["nblk"]
    nlp = cfg["nlp"]
    ep1 = cfg["ep1"]
    SL = cfg["SL"]
    IV = cfg["IV"]
    gb = cfg["gb"]
    glen = cfg["glen"]
    rel_of_chunk = cfg["rel_of_chunk"]

    nc = bacc.Bacc(None, target_bir_lowering=False, debug=False)

    xsT = nc.declare_dram_parameter("xsT", [128, ep1], BF16, isOutput=False)
    wsb = nc.declare_dram_parameter("wsb", [128, (R + 1) * 128], BF16,
                                    isOutput=False)
    xmt = nc.declare_dram_parameter("xmt", [128, nlp], BF16, isOutput=False)
    cidx = nc.declare_dram_parameter("cidx", [128, nblk], I32, isOutput=False)
    tcol = nc.declare_dram_parameter("tcol", [128, nblk * SL], BF16,
                                     isOutput=False)
    outT = nc.declare_dram_parameter("outT", [128, nlp], BF16, isOutput=True)

    md = [nc.dram_tensor(f"md{g}", [glen[g], 128], BF16) for g in range(G)]

    # colw[p, s*128 + e] = e  (iota over e, tiled SL times)
    colw_d = nc.inline_tensor(
        np.tile(np.arange(128, dtype=np.float32), (128, SL)),
        name="colw_c")

    gxoff = np.concatenate([[0], np.cumsum(glen)[:-1]]).astype(np.int64)

    with tile.TileContext(nc) as tc:
        with tc.tile_pool(name="const", bufs=1) as constp:
            wsb_t = constp.tile([128, (R + 1) * 128], BF16)
            nc.sync.dma_start(out=wsb_t[:], in_=wsb[:])
            xmt_t = constp.tile([128, nlp], BF16)
            nc.sync.dma_start(out=xmt_t[:], in_=xmt[:])
            cidx_t = constp.tile([128, nblk], I32)
            nc.sync.dma_start(out=cidx_t[:], in_=cidx[:])
            tcol_t = constp.tile([128, nblk * SL], BF16)
            nc.sync.dma_start(out=tcol_t[:], in_=tcol[:])
            colw_f = constp.tile([128, SL * 128], F32)
            nc.sync.dma_start(out=colw_f[:], in_=colw_d[:])
            colw = constp.tile([128, SL * 128], BF16)
            nc.vector.tensor_copy(out=colw[:], in_=colw_f[:])

            with (
                tc.tile_pool(name="xt", bufs=4) as xp,
                tc.tile_pool(name="mds", bufs=3) as mdp,
                tc.tile_pool(name="p1ps", bufs=5, space="PSUM") as p1ps,
                tc.tile_pool(name="p2", bufs=8) as p2,
                tc.tile_pool(name="p2t", bufs=8) as p2t,
                tc.tile_pool(name="io", bufs=2) as iop,
                tc.tile_pool(name="p2ps", bufs=3, space="PSUM") as p2ps,
            ):
                state = {"ncopy": 0, "obt": None, "obn": 0, "obb": -1}
                ready = []    # blocks whose md group is complete

                def flush_ob():
                    # one DMA for the accumulated run of consecutive blocks
                    if state["obn"]:
                        b0 = state["obb"] - state["obn"] + 1
                        nc.scalar.dma_start(
                            out=outT[:, 128 * b0:128 * (b0 + state["obn"])],
                            in_=state["obt"][:, :128 * state["obn"]])
                    state["obt"] = None
                    state["obn"] = 0

                gathered = []

                def emit_gather(b, g):
                    iv = IV
                    mg = p2.tile([128, SL * 128], BF16, tag="mg")
                    nc.gpsimd.indirect_dma_start(
                        out=mg[0:iv, :], out_offset=None, in_=md[g][:, :],
                        in_offset=bass.IndirectOffsetOnAxis(
                            ap=cidx_t[0:iv, b:b + 1], axis=0))
                    tt = p2t.tile([128, SL * 128], BF16, tag="T")
                    tsl = tcol_t[0:iv, b * SL:(b + 1) * SL]
                    tb = tsl.unsqueeze(2).broadcast_to([iv, SL, 128])
                    nc.vector.tensor_tensor(
                        out=tt[0:iv].rearrange("p (s e) -> p s e", e=128),
                        in0=colw[0:iv].rearrange("p (s e) -> p s e", e=128),
                        in1=tb, op=mybir.AluOpType.is_equal)
                    gathered.append((b, mg, tt))

                def emit_mms(b, mg, tt):
                    iv = IV
                    ps = p2ps.tile([128, 128], F32, tag="acc")
                    nc.tensor.matmul(
                        out=ps[:],
                        lhsT=wsb_t[:, R * 128:(R + 1) * 128],
                        rhs=xmt_t[:, 128 * b:128 * (b + 1)],
                        start=True, stop=False)
                    for j in range(SL):
                        nc.tensor.matmul(
                            out=ps[:],
                            lhsT=mg[0:iv, 128 * j:128 * (j + 1)],
                            rhs=tt[0:iv, 128 * j:128 * (j + 1)],
                            start=False, stop=(j == SL - 1))
                    if state["obn"] and b != state["obb"] + 1:
                        flush_ob()
                    if state["obt"] is None:
                        obt = iop.tile([128, 8 * 128], BF16, tag="ob")
                        state["obt"] = obt
                    k = state["obn"]
                    nc.scalar.copy(out=state["obt"][:, 128 * k:128 * (k + 1)],
                                   in_=ps[:])
                    state["obb"] = b
                    state["obn"] = k + 1
                    if state["obn"] == 8:
                        flush_ob()

                def drain(k):
                    # lockstep: one new gather per matmul-set emitted, so
                    # matmuls always lag gathers by the seeded depth
                    nn = 0
                    while ready and gathered and nn < k:
                        emit_gather(*ready.pop(0))
                        emit_mms(*gathered.pop(0))
                        nn += 1
                    while not ready and gathered and nn < k:
                        emit_mms(*gathered.pop(0))
                        nn += 1
                    flush_ob()

                # ---------------- Phase 1: messages ----------------
                slices = [(g, loff, lsz) for g in range(G)
                          for loff, lsz in _splits(glen[g], LDTOK)]
                xts = {}

                def emit_load(i):
                    g, loff, lsz = slices[i]
                    xt = xp.tile([128, LDTOK], BF16, tag="xt")
                    nc.scalar.dma_start(
                        out=xt[:, :lsz],
                        in_=xsT[:, gxoff[g] + loff:gxoff[g] + loff + lsz])
                    xts[i] = xt

                emit_load(0)
                for i, (g, loff, lsz) in enumerate(slices):
                    roc = rel_of_chunk[g]
                    # prefetch the next slice's load ahead of this slice's
                    # compute so the scalar-ring dispatch isn't queued
                    # behind this slice's copies
                    if i + 1 < len(slices):
                        emit_load(i + 1)
                    xt = xts.pop(i)
                    if True:
                        for goff, gsz in _splits(lsz, MDG * 128):
                            ms = mdp.tile([128, MDG * 128], BF16, tag="ms")
                            for poff, psz in _splits(gsz, PS4 * 128):
                                ps = p1ps.tile([128, PS4 * 128], F32,
                                               tag="ps")
                                for j in range(psz // 128):
                                    ch = (loff + goff + poff) // 128 + j
                                    r = roc[ch]
                                    co = goff + poff + 128 * j
                                    nc.tensor.matmul(
                                        out=ps[:, 128 * j:128 * (j + 1)],
                                        lhsT=xt[:, co:co + 128],
                                        rhs=wsb_t[:, 128 * r:128 * (r + 1)],
                                        start=True, stop=True)
                                state["ncopy"] += 1
                                if state["ncopy"] % 3 == 0:
                                    nc.vector.tensor_copy(
                                        out=ms[:, poff:poff + psz],
                                        in_=ps[:, :psz])
                                else:
                                    nc.scalar.copy(
                                        out=ms[:, poff:poff + psz],
                                        in_=ps[:, :psz])
                            r0 = loff + goff
                            mdv = md[g][r0:r0 + gsz, :]
                            nc.sync.dma_start(
                                out=mdv.rearrange("(w p j) c -> p w (j c)",
                                                  p=128, j=2),
                                in_=ms[:, :gsz].rearrange("p (w k) -> p w k",
                                                          k=256))
                            drain(QUOTA)
                    # group g's messages fully emitted -> its blocks ready;
                    # pre-emit the first few gathers so the matmuls that
                    # follow never wait on a just-issued gather
                    if i + 1 == len(slices) or slices[i + 1][0] != g:
                        ready.extend((b, g)
                                     for b in range(gb[g], gb[g + 1]))
                        for _ in range(SEED):
                            if ready:
                                emit_gather(*ready.pop(0))

                # ---------------- Phase 2 tail ----------------
                drain(len(ready) + len(gathered))
                flush_ob()

    nc.finalize()
    return nc


_PROGRAM_CACHE = {}


def _get_program(cfg):
    key = tuple(sorted((k, str(v)) for k, v in cfg.items()))
    if key not in _PROGRAM_CACHE:
        _PROGRAM_CACHE[key] = build_program(cfg)
    return _PROGRAM_CACHE[key]


def kernel(x, node_keep_mask, source, target, edge_type, edge_weights,
           bases, relation_base_weights):
    per_core, cfg = host_prep(x, node_keep_mask, source, target, edge_type,
                              edge_weights, bases, relation_base_weights)
    nc = _get_program(cfg)
    res = run_bass_kernel_spmd(nc, per_core, list(range(NCORE)))
    nl = cfg["nl"]
    out = np.empty((cfg["n"], 128), np.float32)
    for c in range(NCORE):
        out[c * nl:(c + 1) * nl] = \
            res.results[c]["outT"][:, :nl].T.astype(np.float32)
    return out
